# revision 15
# baseline (speedup 1.0000x reference)
"""Trainium2 Bass kernel for multi-head attention (b=4, n=2048, d=512, h=8, dk=dv=64).

Sharding: 8 cores = 4 batches x 2 query-halves. Each core computes K/V for its
full batch sequence (2048) and attention outputs for its 1024 query rows.
No collectives needed; host stacks the per-core [1024, 512] outputs.

Per-core dataflow:
  x^T [512, 2048] staged in SBUF as bf16; projections (bf16 MMs, f32 PSUM)
  run mostly up front, emission-ordered against the HBM input stream.
  Q/K projections are head-PAIR packed: one [128 = h_even dims | h_odd dims]
  PSUM tile per pair covers two heads per moving stream (halved MM columns,
  unreplicated wq/wk).  Per-head S^T keeps full 128x128 stationaries via the
  zero-half trick: qt_h = [q+bias; 0] (or flipped), kt_h live half = K^T, dead
  half zeroed, so S^T = kt_h.T @ qt_h contracts the zero half away.  All
  PSUM->SBUF copies stay partition-aligned.  S^T/PV stay f32r/bf16 as in the
  baseline; exp on ScalarE from PSUM per [128,1024] chunk (the phase pacer),
  PV runs 3 chunks behind S^T so it never stalls on exp; K-projections for
  later pairs are interleaved mid-head to keep the PE HAM-warm.
"""
import numpy as np

B, N, MODEL = 4, 2048, 512
H, DK = 8, 64
SCALE = DK ** -0.5
NI = 1024           # query rows per core
NCH = MODEL // 128  # model-dim chunks
NJC = N // 128      # key/value chunks
NHP = H // 2        # head pairs
LOOK = 3            # PV chunk lookahead behind S^T

_COMPILED = None


def _build():
    import concourse.bass as bass
    from concourse import bacc
    import concourse.mybir as mybir
    import concourse.tile as tile

    F32 = mybir.dt.float32
    F32R = mybir.dt.float32r
    BF16 = mybir.dt.bfloat16
    EXP = mybir.ActivationFunctionType.Exp

    nc = bacc.Bacc("TRN2", target_bir_lowering=False, debug=False, num_devices=8)
    xt_in = nc.dram_tensor("xt", [MODEL, N], BF16, kind="ExternalInput")
    wq_in = nc.dram_tensor("wq", [MODEL, MODEL], BF16, kind="ExternalInput")
    wk_in = nc.dram_tensor("wk", [MODEL, MODEL], BF16, kind="ExternalInput")
    wv_in = nc.dram_tensor("wv", [MODEL, MODEL], BF16, kind="ExternalInput")
    relb_in = nc.dram_tensor("relb", [128, NHP], F32, kind="ExternalInput")
    wo_in = nc.dram_tensor("wo", [MODEL, MODEL], F32R, kind="ExternalInput")
    bo_in = nc.dram_tensor("bo", [1, MODEL], F32, kind="ExternalInput")
    onesb_in = nc.dram_tensor("onesb", [128, NJC * H], BF16, kind="ExternalInput")
    y_out = nc.dram_tensor("y", [NI, MODEL], F32, kind="ExternalOutput")

    with tile.TileContext(nc) as tc:
        with (
            tc.tile_pool(name="w", bufs=1) as wp,
            tc.tile_pool(name="acts", bufs=1) as ap,
            tc.tile_pool(name="big", bufs=3, space="PSUM") as ps,
        ):
            # ---------- persistent tiles ----------
            wo = wp.tile([128, NCH, MODEL], F32R, tag="wo")
            bo = wp.tile([1, MODEL], F32, tag="bo")
            bo_b = wp.tile([128, MODEL], F32, tag="bo_b")
            vv_a = ap.tile([128, NJC // 2, H * 65], BF16, tag="vva")
            vv_b = ap.tile([128, NJC // 2, H * 65], BF16, tag="vvb")
            def vvt(jc):
                return (vv_a if jc < NJC // 2 else vv_b)[:, jc % (NJC // 2)]
            relb = ap.tile([128, NHP], F32, tag="relb")
            outt = ap.tile([128, NCH, NI], F32R, tag="outt")
            kt = ap.tile([128, H, NJC, 128], F32R, tag="kt")
            qt = ap.tile([128, H, NI], F32R, tag="qt")

            def r3(d):
                return d[:].rearrange("(c p) n -> p c n", p=128)

            dma_n = [0]
            def dma(out, in_):
                engs = (nc.sync, nc.gpsimd, nc.scalar)
                engs[dma_n[0] % 3].dma_start(out=out, in_=in_)
                dma_n[0] += 1

            with tc.tile_pool(name="proj", bufs=1) as pp:
                xt0 = pp.tile([128, NCH, 512], BF16, tag="xt0")
                xt1 = pp.tile([128, NCH, 512], BF16, tag="xt1")
                xt2 = pp.tile([128, NCH, 512], BF16, tag="xt2")
                xt3 = pp.tile([128, NCH, 512], BF16, tag="xt3")
                xts = [xt0, xt1, xt2, xt3]
                wq = pp.tile([128, NCH, MODEL], BF16, tag="wq")
                wk = pp.tile([128, NCH, MODEL], BF16, tag="wk")
                wv = pp.tile([128, NCH, MODEL], BF16, tag="wv")
                onesb_t = pp.tile([128, NJC * H], BF16, tag="onesb")

                # ---- DMA emission in global priority order (round-robin) ----
                xsrc = r3(xt_in)
                def dma_x(q):
                    for chh in range(2):
                        dma(xts[q][:, chh * 2:(chh + 1) * 2, :],
                            xsrc[:, chh * 2:(chh + 1) * 2, q * 512:(q + 1) * 512])
                for ch in range(NCH):
                    dma(wv[:, ch], r3(wv_in)[:, ch])
                dma_x(0)
                for ch in range(NCH):
                    dma(wq[:, ch], r3(wq_in)[:, ch])
                dma(relb[:], relb_in[:])
                dma_x(1)
                dma_x(2)
                for ch in range(NCH):
                    dma(wk[:, ch], r3(wk_in)[:, ch])
                dma_x(3)
                for ch in range(NCH):
                    dma(wo[:, ch], r3(wo_in)[:, ch])
                dma(bo[:], bo_in[:])
                dma(onesb_t[:], onesb_in[:])
                nc.gpsimd.partition_broadcast(bo_b[:], bo[:])
                # ones columns of V_aug: contiguous DMA to scratch, strided copy
                for vh in range(2):
                    nc.vector.tensor_copy(
                        (vv_a if vh == 0 else vv_b)[:]
                        .rearrange("p j (h e) -> p (j h) e", e=65)[:, :, 64:65],
                        onesb_t[:, vh * NJC * H // 2:(vh + 1) * NJC * H // 2]
                        .rearrange("p (n o) -> p n o", o=1))
                # zero qt/kt up front (the dead halves of the zero-half trick
                # must be finite; live halves are overwritten by the Q/K
                # projection drains)
                for h in range(H):
                    nc.vector.memset(qt[:, h, :].bitcast(F32), 0.0)
                    nc.vector.memset(kt[:, h].bitcast(F32), 0.0)

                def xtv(ch, start, size):
                    t = xts[start // 512]
                    off = start % 512
                    assert off + size <= 512
                    return t[:, ch, off:off + size]

                # ---- compute emission, ordered to match DMA arrival ----
                def emit_v(jcs):
                    for jc in jcs:
                        v_ps = ps.tile([128, NI], F32, tag="big")
                        for ch in range(NCH):
                            nc.tensor.matmul(v_ps[:, 0:MODEL],
                                             xtv(ch, jc * 128, 128),
                                             wv[:, ch],
                                             start=(ch == 0), stop=(ch == NCH - 1))
                        nc.vector.tensor_copy(
                            vvt(jc).rearrange("p (h e) -> p h e", e=65)[:, :, 0:64],
                            v_ps[:, 0:MODEL].rearrange("p (h e) -> p h e", e=64))

                def emit_q(hp):
                    q_ps = ps.tile([128, NI], F32, tag="big")
                    for ib in range(2):
                        for ch in range(NCH):
                            nc.tensor.matmul(
                                q_ps[:, ib * 512:(ib + 1) * 512],
                                wq[:, ch, hp * 128:(hp + 1) * 128],
                                xtv(ch, ib * 512, 512),
                                start=(ch == 0), stop=(ch == NCH - 1))
                    nc.vector.tensor_scalar_add(
                        qt[0:64, 2 * hp, :], q_ps[0:64, :], relb[0:64, hp:hp + 1])
                    nc.vector.tensor_scalar_add(
                        qt[64:128, 2 * hp + 1, :], q_ps[64:128, :],
                        relb[64:128, hp:hp + 1])

                def emit_k(hp, jb):
                    k_ps = ps.tile([128, NI], F32, tag="big")
                    for sb in range(2):
                        off = jb * NI + sb * 512
                        for ch in range(NCH):
                            nc.tensor.matmul(
                                k_ps[:, sb * 512:(sb + 1) * 512],
                                wk[:, ch, hp * 128:(hp + 1) * 128],
                                xtv(ch, off, 512),
                                start=(ch == 0), stop=(ch == NCH - 1))
                    jcs = slice(jb * 8, jb * 8 + 8)
                    src = k_ps[:].rearrange("p (j m) -> p j m", m=128)
                    nc.vector.tensor_copy(kt[0:64, 2 * hp, jcs, :], src[0:64])
                    nc.vector.tensor_copy(kt[64:128, 2 * hp + 1, jcs, :],
                                          src[64:128])

                emit_v(range(0, 4))
                emit_q(0)
                emit_q(1)
                emit_v(range(4, 8))
                emit_q(2)
                emit_q(3)
                emit_v(range(8, 12))
                emit_k(0, 0)
                emit_v(range(12, 16))
                emit_k(0, 1)

                # ------ attention: per head S^T -> exp -> PV (PV LOOK chunks
                # behind); K-proj for later pairs interleaved to stay warm ----
                with tc.tile_pool(name="pt", bufs=5) as ptp, \
                     tc.tile_pool(name="pv", bufs=1, space="PSUM") as pvp, \
                     tc.tile_pool(name="norm", bufs=2) as np_, \
                     tc.tile_pool(name="ysb", bufs=2) as yp_sb:
                    for h in range(H):
                        hp = h // 2
                        pv_t = pvp.tile([65, NI], F32, tag="pv")
                        pts = {}

                        def emit_pv(jc):
                            pt = pts.pop(jc)
                            for ih in range(2):
                                nc.tensor.matmul(
                                    pv_t[:, ih * 512:(ih + 1) * 512],
                                    vvt(jc)[:, h * 65:(h + 1) * 65],
                                    pt[:, ih * 512:(ih + 1) * 512],
                                    start=(jc == 0), stop=(jc == NJC - 1))

                        for jc in range(NJC):
                            st = ps.tile([128, NI], F32, tag="big")
                            for ih in range(2):
                                nc.tensor.matmul(
                                    st[:, ih * 512:(ih + 1) * 512],
                                    kt[:, h, jc],
                                    qt[:, h, ih * 512:(ih + 1) * 512],
                                    start=True, stop=True)
                            pt = ptp.tile([128, NI], BF16, tag="pt")
                            pts[jc] = pt
                            nc.scalar.activation(pt[:], st[:], EXP, scale=1.0)
                            if jc == 7 and h < 6:
                                emit_k(1 + h // 2, h % 2)
                            if jc >= LOOK:
                                emit_pv(jc - LOOK)
                        for jc in range(NJC - LOOK, NJC):
                            emit_pv(jc)

                        base = (h % 2) * 64
                        den = np_.tile([1, NI], F32, tag="den")
                        nc.vector.tensor_copy(den[:], pv_t[64:65, :])
                        rrow = np_.tile([1, NI], F32, tag="rrow")
                        nc.vector.reciprocal_approx_fast(rrow[:], den[:])
                        rb = np_.tile([64, NI], F32, tag="rb")
                        nc.gpsimd.partition_broadcast(rb[:], rrow[:])
                        nc.vector.tensor_tensor(
                            out=outt[base:base + 64, hp, :],
                            in0=pv_t[0:64, :], in1=rb[:],
                            op=mybir.AluOpType.mult)

                    # ---------- output projection ----------
                    for ib in range(NI // 128):
                        y_ps = ps.tile([128, NI], F32, tag="big")
                        for ch in range(NCH):
                            nc.tensor.matmul(y_ps[:, 0:MODEL],
                                             outt[:, ch, ib * 128:(ib + 1) * 128],
                                             wo[:, ch],
                                             start=(ch == 0), stop=(ch == NCH - 1))
                        y_sb = yp_sb.tile([128, MODEL], F32, tag="ysb")
                        nc.vector.tensor_tensor(out=y_sb[:], in0=y_ps[:, 0:MODEL],
                                                in1=bo_b[:],
                                                op=mybir.AluOpType.add)
                        dma(y_out[ib * 128:(ib + 1) * 128, :], y_sb[:])

    nc.compile()
    return nc


def _get_compiled():
    global _COMPILED
    if _COMPILED is None:
        _COMPILED = _build()
    return _COMPILED


def kernel(x, Wq, Wk, Wv, Wo, bo, rel_content_bias, _trace=False):
    from concourse.bass_utils import run_bass_kernel_spmd
    import ml_dtypes

    nc = _get_compiled()
    BF = ml_dtypes.bfloat16

    x = np.asarray(x, dtype=np.float32)
    Wq = np.asarray(Wq, dtype=np.float32)
    Wk = np.asarray(Wk, dtype=np.float32)
    Wv = np.asarray(Wv, dtype=np.float32)
    Wo = np.asarray(Wo, dtype=np.float32)
    bo = np.asarray(bo, dtype=np.float32)
    bias = np.asarray(rel_content_bias, dtype=np.float32).reshape(H, DK)

    wq_b = (Wq * SCALE).astype(BF)
    wk_b = Wk.astype(BF)
    wv_b = Wv.astype(BF)
    # relb packed per head pair: rows 0:64 = even head bias, 64:128 = odd head
    relb = np.ascontiguousarray(
        bias.reshape(NHP, 2, DK).transpose(1, 2, 0).reshape(128, NHP))
    onesb = np.ones((128, NJC * H), BF)
    shared = {"wq": wq_b, "wk": wk_b, "wv": wv_b, "relb": relb, "wo": Wo,
              "bo": bo[None, :], "onesb": onesb}

    in_maps = []
    for c in range(8):
        b, half = c // 2, c % 2
        xt = np.ascontiguousarray(x[b].T)              # [512, 2048]
        if half:
            xt = np.ascontiguousarray(np.roll(xt, -NI, axis=1))
        in_maps.append({"xt": xt.astype(BF), **shared})

    res = run_bass_kernel_spmd(nc, in_maps, core_ids=list(range(8)),
                               trace=_trace)
    out = np.empty((B, N, MODEL), np.float32)
    for c in range(8):
        b, half = c // 2, c % 2
        out[b, half * NI:(half + 1) * NI, :] = res.results[c]["y"]
    if _trace:
        return out, res
    return out


# revision 16
# speedup vs baseline: 1.1114x; 1.1114x over previous
"""Trainium2 Bass kernel for multi-head attention (b=4, n=2048, d=512, h=8, dk=dv=64).

Sharding: 8 cores = 4 batches x 2 query-halves. Each core computes K/V for its
full batch sequence (2048) and attention outputs for its 1024 query rows.
No collectives needed; host stacks the per-core [1024, 512] outputs.

Per-core dataflow:
  x^T [512, 2048] staged in SBUF as bf16; projections (bf16 MMs, f32 PSUM)
  run mostly up front, emission-ordered against the HBM input stream.
  Q/K projections are head-PAIR packed: one [128 = h_even dims | h_odd dims]
  PSUM tile per pair covers two heads per moving stream (halved MM columns,
  unreplicated wq/wk).  Per-head S^T keeps full 128x128 stationaries via the
  zero-half trick: qt_h = [q+bias; 0] (or flipped), kt_h live half = K^T, dead
  half zeroed, so S^T = kt_h.T @ qt_h contracts the zero half away.  All
  PSUM->SBUF copies stay partition-aligned.  S^T/PV stay f32r/bf16 as in the
  baseline; exp on ScalarE from PSUM per [128,1024] chunk (the phase pacer),
  PV runs 3 chunks behind S^T so it never stalls on exp; K-projections for
  later pairs are interleaved mid-head to keep the PE HAM-warm.
"""
import numpy as np

B, N, MODEL = 4, 2048, 512
H, DK = 8, 64
SCALE = DK ** -0.5
NI = 1024           # query rows per core
NCH = MODEL // 128  # model-dim chunks
NJC = N // 128      # key/value chunks
NHP = H // 2        # head pairs
LOOK = 3            # PV chunk lookahead behind S^T

_COMPILED = None


def _build():
    import concourse.bass as bass
    from concourse import bacc
    import concourse.mybir as mybir
    import concourse.tile as tile

    F32 = mybir.dt.float32
    F32R = mybir.dt.float32r
    BF16 = mybir.dt.bfloat16
    EXP = mybir.ActivationFunctionType.Exp

    nc = bacc.Bacc("TRN2", target_bir_lowering=False, debug=False, num_devices=8)
    xt_in = nc.dram_tensor("xt", [MODEL, N], BF16, kind="ExternalInput")
    wq_in = nc.dram_tensor("wq", [MODEL, MODEL], BF16, kind="ExternalInput")
    wk_in = nc.dram_tensor("wk", [MODEL, MODEL], BF16, kind="ExternalInput")
    wv_in = nc.dram_tensor("wv", [MODEL, MODEL], BF16, kind="ExternalInput")
    relb_in = nc.dram_tensor("relb", [128, NHP], F32, kind="ExternalInput")
    wo_in = nc.dram_tensor("wo", [MODEL, MODEL], F32R, kind="ExternalInput")
    bo_in = nc.dram_tensor("bo", [1, MODEL], F32, kind="ExternalInput")
    onesb_in = nc.dram_tensor("onesb", [128, NJC * H], BF16, kind="ExternalInput")
    y_out = nc.dram_tensor("y", [NI, MODEL], F32, kind="ExternalOutput")

    with tile.TileContext(nc) as tc:
        with (
            tc.tile_pool(name="w", bufs=1) as wp,
            tc.tile_pool(name="acts", bufs=1) as ap,
            tc.tile_pool(name="big", bufs=3, space="PSUM") as ps,
        ):
            # ---------- persistent tiles ----------
            wo = wp.tile([128, NCH, MODEL], F32R, tag="wo")
            bo = wp.tile([1, MODEL], F32, tag="bo")
            bo_b = wp.tile([128, MODEL], F32, tag="bo_b")
            vv_a = ap.tile([128, NJC // 2, H * 65], BF16, tag="vva")
            vv_b = ap.tile([128, NJC // 2, H * 65], BF16, tag="vvb")
            def vvt(jc):
                return (vv_a if jc < NJC // 2 else vv_b)[:, jc % (NJC // 2)]
            relb = ap.tile([128, NHP], F32, tag="relb")
            outt = ap.tile([128, NCH, NI], F32R, tag="outt")
            kt = ap.tile([128, NHP, NJC, 128], F32R, tag="kt")
            qt = ap.tile([128, H, NI], F32R, tag="qt")

            def r3(d):
                return d[:].rearrange("(c p) n -> p c n", p=128)

            dma_n = [0]
            def dma(out, in_):
                engs = (nc.sync, nc.gpsimd, nc.scalar)
                engs[dma_n[0] % 3].dma_start(out=out, in_=in_)
                dma_n[0] += 1

            with tc.tile_pool(name="proj", bufs=1) as pp:
                xt0 = pp.tile([128, NCH, 512], BF16, tag="xt0")
                xt1 = pp.tile([128, NCH, 512], BF16, tag="xt1")
                xt2 = pp.tile([128, NCH, 512], BF16, tag="xt2")
                xt3 = pp.tile([128, NCH, 512], BF16, tag="xt3")
                xts = [xt0, xt1, xt2, xt3]
                wq = pp.tile([128, NCH, MODEL], BF16, tag="wq")
                wk = pp.tile([128, NCH, MODEL], BF16, tag="wk")
                wv = pp.tile([128, NCH, MODEL], BF16, tag="wv")
                onesb_t = pp.tile([128, NJC * H], BF16, tag="onesb")

                # ---- DMA emission in global priority order (round-robin) ----
                xsrc = r3(xt_in)
                def dma_x(q):
                    for chh in range(2):
                        dma(xts[q][:, chh * 2:(chh + 1) * 2, :],
                            xsrc[:, chh * 2:(chh + 1) * 2, q * 512:(q + 1) * 512])
                for ch in range(NCH):
                    dma(wv[:, ch], r3(wv_in)[:, ch])
                dma_x(0)
                for ch in range(NCH):
                    dma(wq[:, ch], r3(wq_in)[:, ch])
                dma(relb[:], relb_in[:])
                dma_x(1)
                dma_x(2)
                for ch in range(NCH):
                    dma(wk[:, ch], r3(wk_in)[:, ch])
                dma_x(3)
                for ch in range(NCH):
                    dma(wo[:, ch], r3(wo_in)[:, ch])
                dma(bo[:], bo_in[:])
                dma(onesb_t[:], onesb_in[:])
                nc.gpsimd.partition_broadcast(bo_b[:], bo[:])
                # ones columns of V_aug: contiguous DMA to scratch, strided copy
                for vh in range(2):
                    nc.vector.tensor_copy(
                        (vv_a if vh == 0 else vv_b)[:]
                        .rearrange("p j (h e) -> p (j h) e", e=65)[:, :, 64:65],
                        onesb_t[:, vh * NJC * H // 2:(vh + 1) * NJC * H // 2]
                        .rearrange("p (n o) -> p n o", o=1))
                # zero qt up front (the dead half of the zero-half trick must
                # be zero to mask the other head's K rows in the shared kt;
                # live halves are overwritten by the Q projection drains)
                for h in range(H):
                    nc.gpsimd.memset(qt[:, h, :].bitcast(F32), 0.0)

                def xtv(ch, start, size):
                    t = xts[start // 512]
                    off = start % 512
                    assert off + size <= 512
                    return t[:, ch, off:off + size]

                # ---- compute emission, ordered to match DMA arrival ----
                def emit_v(jcs):
                    for jc in jcs:
                        v_ps = ps.tile([128, NI], F32, tag="big")
                        for ch in range(NCH):
                            nc.tensor.matmul(v_ps[:, 0:MODEL],
                                             xtv(ch, jc * 128, 128),
                                             wv[:, ch],
                                             start=(ch == 0), stop=(ch == NCH - 1))
                        nc.vector.tensor_copy(
                            vvt(jc).rearrange("p (h e) -> p h e", e=65)[:, :, 0:64],
                            v_ps[:, 0:MODEL].rearrange("p (h e) -> p h e", e=64))

                def emit_q(hp):
                    q_ps = ps.tile([128, NI], F32, tag="big")
                    for ib in range(2):
                        for ch in range(NCH):
                            nc.tensor.matmul(
                                q_ps[:, ib * 512:(ib + 1) * 512],
                                wq[:, ch, hp * 128:(hp + 1) * 128],
                                xtv(ch, ib * 512, 512),
                                start=(ch == 0), stop=(ch == NCH - 1))
                    nc.vector.tensor_scalar_add(
                        qt[0:64, 2 * hp, :], q_ps[0:64, :], relb[0:64, hp:hp + 1])
                    nc.vector.tensor_scalar_add(
                        qt[64:128, 2 * hp + 1, :], q_ps[64:128, :],
                        relb[64:128, hp:hp + 1])

                def emit_k(hp, jb):
                    k_ps = ps.tile([128, NI], F32, tag="big")
                    for sb in range(2):
                        off = jb * NI + sb * 512
                        for ch in range(NCH):
                            nc.tensor.matmul(
                                k_ps[:, sb * 512:(sb + 1) * 512],
                                wk[:, ch, hp * 128:(hp + 1) * 128],
                                xtv(ch, off, 512),
                                start=(ch == 0), stop=(ch == NCH - 1))
                    jcs = slice(jb * 8, jb * 8 + 8)
                    src = k_ps[:].rearrange("p (j m) -> p j m", m=128)
                    nc.vector.tensor_copy(kt[0:64, hp, jcs, :], src[0:64])
                    nc.vector.tensor_copy(kt[64:128, hp, jcs, :], src[64:128])

                emit_v(range(0, 4))
                emit_q(0)
                emit_q(1)
                emit_v(range(4, 8))
                emit_q(2)
                emit_q(3)
                emit_v(range(8, 12))
                emit_k(0, 0)
                emit_v(range(12, 16))
                emit_k(0, 1)

                # ------ attention: per head S^T -> exp -> PV (PV LOOK chunks
                # behind); K-proj for later pairs interleaved to stay warm ----
                with tc.tile_pool(name="pt", bufs=8) as ptp, \
                     tc.tile_pool(name="pv", bufs=1, space="PSUM") as pvp, \
                     tc.tile_pool(name="norm", bufs=2) as np_, \
                     tc.tile_pool(name="ysb", bufs=2) as yp_sb:
                    for h in range(H):
                        hp = h // 2
                        pv_t = pvp.tile([65, NI], F32, tag="pv")
                        pts = {}

                        def emit_pv(jc):
                            pt = pts.pop(jc)
                            for ih in range(2):
                                nc.tensor.matmul(
                                    pv_t[:, ih * 512:(ih + 1) * 512],
                                    vvt(jc)[:, h * 65:(h + 1) * 65],
                                    pt[:, ih * 512:(ih + 1) * 512],
                                    start=(jc == 0), stop=(jc == NJC - 1))

                        for jc in range(NJC):
                            st = ps.tile([128, NI], F32, tag="big")
                            for ih in range(2):
                                nc.tensor.matmul(
                                    st[:, ih * 512:(ih + 1) * 512],
                                    kt[:, hp, jc],
                                    qt[:, h, ih * 512:(ih + 1) * 512],
                                    start=True, stop=True)
                            pt = ptp.tile([128, NI], BF16, tag="pt")
                            pts[jc] = pt
                            nc.scalar.activation(pt[:], st[:], EXP, scale=1.0)
                            if jc == 7 and h < 6:
                                emit_k(1 + h // 2, h % 2)
                            if jc >= LOOK:
                                emit_pv(jc - LOOK)
                        for jc in range(NJC - LOOK, NJC):
                            emit_pv(jc)

                        base = (h % 2) * 64
                        den = np_.tile([1, NI], F32, tag="den")
                        nc.vector.tensor_copy(den[:], pv_t[64:65, :])
                        rrow = np_.tile([1, NI], F32, tag="rrow")
                        nc.vector.reciprocal_approx_fast(rrow[:], den[:])
                        rb = np_.tile([64, NI], F32, tag="rb")
                        nc.gpsimd.partition_broadcast(rb[:], rrow[:])
                        nc.vector.tensor_tensor(
                            out=outt[base:base + 64, hp, :],
                            in0=pv_t[0:64, :], in1=rb[:],
                            op=mybir.AluOpType.mult)

                    # ---------- output projection ----------
                    for ib in range(NI // 128):
                        y_ps = ps.tile([128, NI], F32, tag="big")
                        for ch in range(NCH):
                            nc.tensor.matmul(y_ps[:, 0:MODEL],
                                             outt[:, ch, ib * 128:(ib + 1) * 128],
                                             wo[:, ch],
                                             start=(ch == 0), stop=(ch == NCH - 1))
                        y_sb = yp_sb.tile([128, MODEL], F32, tag="ysb")
                        nc.vector.tensor_tensor(out=y_sb[:], in0=y_ps[:, 0:MODEL],
                                                in1=bo_b[:],
                                                op=mybir.AluOpType.add)
                        dma(y_out[ib * 128:(ib + 1) * 128, :], y_sb[:])

    nc.compile()
    return nc


def _get_compiled():
    global _COMPILED
    if _COMPILED is None:
        _COMPILED = _build()
    return _COMPILED


def kernel(x, Wq, Wk, Wv, Wo, bo, rel_content_bias, _trace=False):
    from concourse.bass_utils import run_bass_kernel_spmd
    import ml_dtypes

    nc = _get_compiled()
    BF = ml_dtypes.bfloat16

    x = np.asarray(x, dtype=np.float32)
    Wq = np.asarray(Wq, dtype=np.float32)
    Wk = np.asarray(Wk, dtype=np.float32)
    Wv = np.asarray(Wv, dtype=np.float32)
    Wo = np.asarray(Wo, dtype=np.float32)
    bo = np.asarray(bo, dtype=np.float32)
    bias = np.asarray(rel_content_bias, dtype=np.float32).reshape(H, DK)

    wq_b = (Wq * SCALE).astype(BF)
    wk_b = Wk.astype(BF)
    wv_b = Wv.astype(BF)
    # relb packed per head pair: rows 0:64 = even head bias, 64:128 = odd head
    relb = np.ascontiguousarray(
        bias.reshape(NHP, 2, DK).transpose(1, 2, 0).reshape(128, NHP))
    onesb = np.ones((128, NJC * H), BF)
    shared = {"wq": wq_b, "wk": wk_b, "wv": wv_b, "relb": relb, "wo": Wo,
              "bo": bo[None, :], "onesb": onesb}

    in_maps = []
    for c in range(8):
        b, half = c // 2, c % 2
        xt = np.ascontiguousarray(x[b].T)              # [512, 2048]
        if half:
            xt = np.ascontiguousarray(np.roll(xt, -NI, axis=1))
        in_maps.append({"xt": xt.astype(BF), **shared})

    res = run_bass_kernel_spmd(nc, in_maps, core_ids=list(range(8)),
                               trace=_trace)
    out = np.empty((B, N, MODEL), np.float32)
    for c in range(8):
        b, half = c // 2, c % 2
        out[b, half * NI:(half + 1) * NI, :] = res.results[c]["y"]
    if _trace:
        return out, res
    return out


# revision 19
# speedup vs baseline: 1.3336x; 1.1999x over previous
"""Trainium2 Bass kernel for multi-head attention (b=4, n=2048, d=512, h=8, dk=dv=64).

Sharding: 8 cores = 4 batches x 2 query-halves. Each core computes K/V for its
full batch sequence (2048) and attention outputs for its 1024 query rows.
No collectives needed; host stacks the per-core [1024, 512] outputs.

Per-core dataflow:
  x^T [512, 2048] staged in SBUF as bf16; projections (bf16 MMs, f32 PSUM)
  run mostly up front, emission-ordered against the HBM input stream.
  Q/K projections are head-PAIR packed: one [128 = h_even dims | h_odd dims]
  PSUM tile per pair covers two heads per moving stream (halved MM columns,
  unreplicated wq/wk).  Per-head S^T keeps full 128x128 stationaries via the
  zero-half trick: qt_h = [q+bias; 0] (or flipped), kt_h live half = K^T, dead
  half zeroed, so S^T = kt_h.T @ qt_h contracts the zero half away.  All
  PSUM->SBUF copies stay partition-aligned.  S^T/PV stay f32r/bf16 as in the
  baseline; exp on ScalarE from PSUM per [128,1024] chunk (the phase pacer),
  PV runs 3 chunks behind S^T so it never stalls on exp; K-projections for
  later pairs are interleaved mid-head to keep the PE HAM-warm.
"""
import numpy as np

B, N, MODEL = 4, 2048, 512
H, DK = 8, 64
SCALE = DK ** -0.5
NI = 1024           # query rows per core
NCH = MODEL // 128  # model-dim chunks
NJC = N // 128      # key/value chunks
NHP = H // 2        # head pairs
LOOK = 3            # PV chunk lookahead behind S^T

_COMPILED = None


def _build():
    import concourse.bass as bass
    from concourse import bacc
    import concourse.mybir as mybir
    import concourse.tile as tile

    F32 = mybir.dt.float32
    F32R = mybir.dt.float32r
    BF16 = mybir.dt.bfloat16
    F16 = mybir.dt.float16
    EXP = mybir.ActivationFunctionType.Exp

    nc = bacc.Bacc("TRN2", target_bir_lowering=False, debug=False, num_devices=8)
    xt_in = nc.dram_tensor("xt", [MODEL, N], BF16, kind="ExternalInput")
    wq_in = nc.dram_tensor("wq", [MODEL, MODEL], BF16, kind="ExternalInput")
    wk_in = nc.dram_tensor("wk", [MODEL, MODEL], BF16, kind="ExternalInput")
    wv_in = nc.dram_tensor("wv", [MODEL, MODEL], BF16, kind="ExternalInput")
    relb_in = nc.dram_tensor("relb", [128, NHP], F32, kind="ExternalInput")
    wo_in = nc.dram_tensor("wo", [MODEL, MODEL], F32R, kind="ExternalInput")
    bo_in = nc.dram_tensor("bo", [1, MODEL], F32, kind="ExternalInput")
    onesb_in = nc.dram_tensor("onesb", [128, NJC * H], BF16, kind="ExternalInput")
    y_out = nc.dram_tensor("y", [NI, MODEL], F16, kind="ExternalOutput")

    with tile.TileContext(nc) as tc:
        with (
            tc.tile_pool(name="w", bufs=1) as wp,
            tc.tile_pool(name="acts", bufs=1) as ap,
            tc.tile_pool(name="big", bufs=3, space="PSUM") as ps,
        ):
            # ---------- persistent tiles ----------
            wo = wp.tile([128, NCH, MODEL], F32R, tag="wo")
            bo = wp.tile([1, MODEL], F32, tag="bo")
            bo_b = wp.tile([128, MODEL], F32, tag="bo_b")
            vv_a = ap.tile([128, NJC // 2, H * 65], BF16, tag="vva")
            vv_b = ap.tile([128, NJC // 2, H * 65], BF16, tag="vvb")
            def vvt(jc):
                return (vv_a if jc < NJC // 2 else vv_b)[:, jc % (NJC // 2)]
            relb = ap.tile([128, NHP], F32, tag="relb")
            outt = ap.tile([128, NCH, NI], F32R, tag="outt")
            kt = ap.tile([128, NHP, NJC, 128], F32R, tag="kt")
            qt = ap.tile([128, H, NI], F32R, tag="qt")

            def r3(d):
                return d[:].rearrange("(c p) n -> p c n", p=128)

            dma_n = [0]
            def dma(out, in_):
                engs = (nc.sync, nc.gpsimd, nc.scalar)
                engs[dma_n[0] % 3].dma_start(out=out, in_=in_)
                dma_n[0] += 1

            with tc.tile_pool(name="proj", bufs=1) as pp:
                xt0 = pp.tile([128, NCH, 512], BF16, tag="xt0")
                xt1 = pp.tile([128, NCH, 512], BF16, tag="xt1")
                xt2 = pp.tile([128, NCH, 512], BF16, tag="xt2")
                xt3 = pp.tile([128, NCH, 512], BF16, tag="xt3")
                xts = [xt0, xt1, xt2, xt3]
                wq = pp.tile([128, NCH, MODEL], BF16, tag="wq")
                wk = pp.tile([128, NCH, MODEL], BF16, tag="wk")
                wv = pp.tile([128, NCH, MODEL], BF16, tag="wv")
                onesb_t = pp.tile([128, NJC * H], BF16, tag="onesb")

                # ---- DMA emission in global priority order (round-robin) ----
                xsrc = r3(xt_in)
                def dma_x(q):
                    for chh in range(2):
                        dma(xts[q][:, chh * 2:(chh + 1) * 2, :],
                            xsrc[:, chh * 2:(chh + 1) * 2, q * 512:(q + 1) * 512])
                for ch in range(NCH):
                    dma(wv[:, ch], r3(wv_in)[:, ch])
                dma_x(0)
                for ch in range(NCH):
                    dma(wq[:, ch], r3(wq_in)[:, ch])
                dma(relb[:], relb_in[:])
                dma_x(1)
                for ch in range(NCH):
                    dma(wk[:, ch], r3(wk_in)[:, ch])
                dma_x(2)
                dma_x(3)
                for ch in range(NCH):
                    dma(wo[:, ch], r3(wo_in)[:, ch])
                dma(bo[:], bo_in[:])
                dma(onesb_t[:], onesb_in[:])
                nc.gpsimd.partition_broadcast(bo_b[:], bo[:])
                # ones columns of V_aug: contiguous DMA to scratch, strided copy
                for vh in range(2):
                    nc.vector.tensor_copy(
                        (vv_a if vh == 0 else vv_b)[:]
                        .rearrange("p j (h e) -> p (j h) e", e=65)[:, :, 64:65],
                        onesb_t[:, vh * NJC * H // 2:(vh + 1) * NJC * H // 2]
                        .rearrange("p (n o) -> p n o", o=1))
                # zero qt up front (the dead half of the zero-half trick must
                # be zero to mask the other head's K rows in the shared kt;
                # live halves are overwritten by the Q projection drains)
                for h in range(H):
                    nc.gpsimd.memset(qt[:, h, :].bitcast(F32), 0.0)

                def xtv(ch, start, size):
                    t = xts[start // 512]
                    off = start % 512
                    assert off + size <= 512
                    return t[:, ch, off:off + size]

                # ---- compute emission, ordered to match DMA arrival ----
                def emit_v(jcs):
                    for jc in jcs:
                        v_ps = ps.tile([128, NI], F32, tag="big")
                        for ch in range(NCH):
                            nc.tensor.matmul(v_ps[:, 0:MODEL],
                                             xtv(ch, jc * 128, 128),
                                             wv[:, ch],
                                             start=(ch == 0), stop=(ch == NCH - 1))
                        nc.vector.tensor_copy(
                            vvt(jc).rearrange("p (h e) -> p h e", e=65)[:, :, 0:64],
                            v_ps[:, 0:MODEL].rearrange("p (h e) -> p h e", e=64))

                def emit_q(hp):
                    q_ps = ps.tile([128, NI], F32, tag="big")
                    for ib in range(2):
                        for ch in range(NCH):
                            nc.tensor.matmul(
                                q_ps[:, ib * 512:(ib + 1) * 512],
                                wq[:, ch, hp * 128:(hp + 1) * 128],
                                xtv(ch, ib * 512, 512),
                                start=(ch == 0), stop=(ch == NCH - 1))
                    nc.vector.tensor_scalar_add(
                        qt[0:64, 2 * hp, :], q_ps[0:64, :], relb[0:64, hp:hp + 1])
                    nc.vector.tensor_scalar_add(
                        qt[64:128, 2 * hp + 1, :], q_ps[64:128, :],
                        relb[64:128, hp:hp + 1])

                def emit_k(hp, jb):
                    k_ps = ps.tile([128, NI], F32, tag="big")
                    for sb in range(2):
                        off = jb * NI + sb * 512
                        for ch in range(NCH):
                            nc.tensor.matmul(
                                k_ps[:, sb * 512:(sb + 1) * 512],
                                wk[:, ch, hp * 128:(hp + 1) * 128],
                                xtv(ch, off, 512),
                                start=(ch == 0), stop=(ch == NCH - 1))
                    jcs = slice(jb * 8, jb * 8 + 8)
                    src = k_ps[:].rearrange("p (j m) -> p j m", m=128)
                    nc.vector.tensor_copy(kt[0:64, hp, jcs, :], src[0:64])
                    nc.vector.tensor_copy(kt[64:128, hp, jcs, :], src[64:128])

                emit_v(range(0, 4))
                emit_q(0)
                emit_k(0, 0)
                emit_k(0, 1)

                # ------ attention: flat pipeline over 128 S^T chunks (8 heads
                # x 16 j-chunks); PV lags LOOK chunks globally so it never
                # stalls on exp and head boundaries stay seamless; remaining
                # projections interleave at scheduled points to keep PE warm -
                ilv = {1: lambda: emit_v(range(4, 8)),
                       4: lambda: emit_v(range(8, 12)),
                       7: lambda: emit_v(range(12, 16)),
                       10: lambda: emit_q(1),
                       13: lambda: emit_k(1, 0),
                       18: lambda: emit_k(1, 1),
                       30: lambda: emit_q(2),
                       38: lambda: emit_k(2, 0),
                       46: lambda: emit_k(2, 1),
                       62: lambda: emit_q(3),
                       70: lambda: emit_k(3, 0),
                       78: lambda: emit_k(3, 1)}
                with tc.tile_pool(name="pt", bufs=8) as ptp, \
                     tc.tile_pool(name="pv", bufs=1, space="PSUM") as pvp, \
                     tc.tile_pool(name="norm", bufs=2) as np_, \
                     tc.tile_pool(name="ysb", bufs=2) as yp_sb:
                    pts = {}
                    pvs = {}

                    def emit_st(g):
                        h, jc = g // NJC, g % NJC
                        st = ps.tile([128, NI], F32, tag="big")
                        for ih in range(2):
                            nc.tensor.matmul(
                                st[:, ih * 512:(ih + 1) * 512],
                                kt[:, h // 2, jc],
                                qt[:, h, ih * 512:(ih + 1) * 512],
                                start=True, stop=True)
                        pt = ptp.tile([128, NI], BF16, tag="pt")
                        pts[g] = pt
                        nc.scalar.activation(pt[:], st[:], EXP, scale=1.0)

                    def emit_pv(g):
                        h, jc = g // NJC, g % NJC
                        if jc == 0:
                            pv_t = pvp.tile([65, NI], F32, tag="pv", name="pv_t")
                            pvs[h] = pv_t
                        else:
                            pv_t = pvs[h]
                        pt = pts.pop(g)
                        for ih in range(2):
                            nc.tensor.matmul(
                                pv_t[:, ih * 512:(ih + 1) * 512],
                                vvt(jc)[:, h * 65:(h + 1) * 65],
                                pt[:, ih * 512:(ih + 1) * 512],
                                start=(jc == 0), stop=(jc == NJC - 1))
                        if jc == NJC - 1:
                            emit_norm(h)

                    def emit_norm(h):
                        hp, base = h // 2, (h % 2) * 64
                        pv_t = pvs.pop(h)
                        den = np_.tile([1, NI], F32, tag="den")
                        nc.vector.tensor_copy(den[:], pv_t[64:65, :])
                        rrow = np_.tile([1, NI], F32, tag="rrow")
                        nc.vector.reciprocal_approx_fast(rrow[:], den[:])
                        rb = np_.tile([64, NI], F32, tag="rb")
                        nc.gpsimd.partition_broadcast(rb[:], rrow[:])
                        nc.vector.tensor_tensor(
                            out=outt[base:base + 64, hp, :],
                            in0=pv_t[0:64, :], in1=rb[:],
                            op=mybir.AluOpType.mult)

                    for g in range(H * NJC + LOOK):
                        if g < H * NJC:
                            emit_st(g)
                        if g in ilv:
                            ilv[g]()
                        if g >= LOOK:
                            emit_pv(g - LOOK)

                    # ---------- output projection ----------
                    for ib in range(NI // 128):
                        y_ps = ps.tile([128, NI], F32, tag="big")
                        for ch in range(NCH):
                            nc.tensor.matmul(y_ps[:, 0:MODEL],
                                             outt[:, ch, ib * 128:(ib + 1) * 128],
                                             wo[:, ch],
                                             start=(ch == 0), stop=(ch == NCH - 1))
                        y_sb = yp_sb.tile([128, MODEL], F16, tag="ysb")
                        nc.vector.tensor_tensor(out=y_sb[:], in0=y_ps[:, 0:MODEL],
                                                in1=bo_b[:],
                                                op=mybir.AluOpType.add)
                        dma(y_out[ib * 128:(ib + 1) * 128, :], y_sb[:])

    nc.compile()
    return nc


def _get_compiled():
    global _COMPILED
    if _COMPILED is None:
        _COMPILED = _build()
    return _COMPILED


def kernel(x, Wq, Wk, Wv, Wo, bo, rel_content_bias, _trace=False):
    from concourse.bass_utils import run_bass_kernel_spmd
    import ml_dtypes

    nc = _get_compiled()
    BF = ml_dtypes.bfloat16

    x = np.asarray(x, dtype=np.float32)
    Wq = np.asarray(Wq, dtype=np.float32)
    Wk = np.asarray(Wk, dtype=np.float32)
    Wv = np.asarray(Wv, dtype=np.float32)
    Wo = np.asarray(Wo, dtype=np.float32)
    bo = np.asarray(bo, dtype=np.float32)
    bias = np.asarray(rel_content_bias, dtype=np.float32).reshape(H, DK)

    wq_b = (Wq * SCALE).astype(BF)
    wk_b = Wk.astype(BF)
    wv_b = Wv.astype(BF)
    # relb packed per head pair: rows 0:64 = even head bias, 64:128 = odd head
    relb = np.ascontiguousarray(
        bias.reshape(NHP, 2, DK).transpose(1, 2, 0).reshape(128, NHP))
    onesb = np.ones((128, NJC * H), BF)
    shared = {"wq": wq_b, "wk": wk_b, "wv": wv_b, "relb": relb, "wo": Wo,
              "bo": bo[None, :], "onesb": onesb}

    in_maps = []
    for c in range(8):
        b, half = c // 2, c % 2
        xt = np.ascontiguousarray(x[b].T)              # [512, 2048]
        if half:
            xt = np.ascontiguousarray(np.roll(xt, -NI, axis=1))
        in_maps.append({"xt": xt.astype(BF), **shared})

    res = run_bass_kernel_spmd(nc, in_maps, core_ids=list(range(8)),
                               trace=_trace)
    out = np.empty((B, N, MODEL), np.float32)
    for c in range(8):
        b, half = c // 2, c % 2
        out[b, half * NI:(half + 1) * NI, :] = res.results[c]["y"]
    if _trace:
        return out, res
    return out


# revision 21
# speedup vs baseline: 1.3573x; 1.0178x over previous
"""Trainium2 Bass kernel for multi-head attention (b=4, n=2048, d=512, h=8, dk=dv=64).

Sharding: 8 cores = 4 batches x 2 query-halves. Each core computes K/V for its
full batch sequence (2048) and attention outputs for its 1024 query rows.
No collectives needed; host stacks the per-core [1024, 512] outputs.

Per-core dataflow:
  x^T [512, 2048] staged in SBUF as bf16; projections (bf16 MMs, f32 PSUM)
  run mostly up front, emission-ordered against the HBM input stream.
  Q/K projections are head-PAIR packed: one [128 = h_even dims | h_odd dims]
  PSUM tile per pair covers two heads per moving stream (halved MM columns,
  unreplicated wq/wk).  Per-head S^T keeps full 128x128 stationaries via the
  zero-half trick: qt_h = [q+bias; 0] (or flipped), kt_h live half = K^T, dead
  half zeroed, so S^T = kt_h.T @ qt_h contracts the zero half away.  All
  PSUM->SBUF copies stay partition-aligned.  S^T/PV stay f32r/bf16 as in the
  baseline; exp on ScalarE from PSUM per [128,1024] chunk (the phase pacer),
  PV runs 3 chunks behind S^T so it never stalls on exp; K-projections for
  later pairs are interleaved mid-head to keep the PE HAM-warm.
"""
import numpy as np

B, N, MODEL = 4, 2048, 512
H, DK = 8, 64
SCALE = DK ** -0.5
NI = 1024           # query rows per core
NCH = MODEL // 128  # model-dim chunks
NJC = N // 128      # key/value chunks
NHP = H // 2        # head pairs
LOOK = 3            # PV chunk lookahead behind S^T

_COMPILED = None


def _build():
    import concourse.bass as bass
    from concourse import bacc
    import concourse.mybir as mybir
    import concourse.tile as tile

    F32 = mybir.dt.float32
    F32R = mybir.dt.float32r
    BF16 = mybir.dt.bfloat16
    F16 = mybir.dt.float16
    EXP = mybir.ActivationFunctionType.Exp

    nc = bacc.Bacc("TRN2", target_bir_lowering=False, debug=False, num_devices=8)
    xt_in = nc.dram_tensor("xt", [MODEL, N], BF16, kind="ExternalInput")
    wq_in = nc.dram_tensor("wq", [MODEL, MODEL], BF16, kind="ExternalInput")
    wk_in = nc.dram_tensor("wk", [MODEL, MODEL], BF16, kind="ExternalInput")
    wv_in = nc.dram_tensor("wv", [MODEL, MODEL], BF16, kind="ExternalInput")
    relb_in = nc.dram_tensor("relb", [128, NHP], F32, kind="ExternalInput")
    wo_in = nc.dram_tensor("wo", [MODEL, MODEL], F32R, kind="ExternalInput")
    bo_in = nc.dram_tensor("bo", [1, MODEL], F32, kind="ExternalInput")
    onesb_in = nc.dram_tensor("onesb", [128, NJC * H], BF16, kind="ExternalInput")
    y_out = nc.dram_tensor("y", [NI, MODEL], F16, kind="ExternalOutput")

    with tile.TileContext(nc) as tc:
        with (
            tc.tile_pool(name="w", bufs=1) as wp,
            tc.tile_pool(name="acts", bufs=1) as ap,
            tc.tile_pool(name="big", bufs=3, space="PSUM") as ps,
        ):
            # ---------- persistent tiles ----------
            wo = wp.tile([128, NCH, MODEL], F32R, tag="wo")
            bo = wp.tile([1, MODEL], F32, tag="bo")
            bo_b = wp.tile([128, MODEL], F32, tag="bo_b")
            vv_a = ap.tile([128, NJC // 2, H * 65], BF16, tag="vva")
            vv_b = ap.tile([128, NJC // 2, H * 65], BF16, tag="vvb")
            def vvt(jc):
                return (vv_a if jc < NJC // 2 else vv_b)[:, jc % (NJC // 2)]
            relb = ap.tile([128, NHP], F32, tag="relb")
            outt = ap.tile([128, NCH, NI], F32R, tag="outt")
            kt = ap.tile([128, NHP, NJC, 128], F32R, tag="kt")
            qt = ap.tile([128, H, NI], F32R, tag="qt")

            def r3(d):
                return d[:].rearrange("(c p) n -> p c n", p=128)

            dma_n = [0]
            def dma(out, in_):
                engs = (nc.sync, nc.gpsimd, nc.scalar)
                engs[dma_n[0] % 3].dma_start(out=out, in_=in_)
                dma_n[0] += 1

            with tc.tile_pool(name="proj", bufs=1) as pp:
                xt0 = pp.tile([128, NCH, 512], BF16, tag="xt0")
                xt1 = pp.tile([128, NCH, 512], BF16, tag="xt1")
                xt2 = pp.tile([128, NCH, 512], BF16, tag="xt2")
                xt3 = pp.tile([128, NCH, 512], BF16, tag="xt3")
                xts = [xt0, xt1, xt2, xt3]
                wq = pp.tile([128, NCH, MODEL], BF16, tag="wq")
                wk = pp.tile([128, NCH, MODEL], BF16, tag="wk")
                wv = pp.tile([128, NCH, MODEL], BF16, tag="wv")
                onesb_t = pp.tile([128, NJC * H], BF16, tag="onesb")

                # ---- DMA emission in global priority order (round-robin) ----
                xsrc = r3(xt_in)
                def dma_x(q):
                    for chh in range(2):
                        dma(xts[q][:, chh * 2:(chh + 1) * 2, :],
                            xsrc[:, chh * 2:(chh + 1) * 2, q * 512:(q + 1) * 512])
                dma(relb[:], relb_in[:])
                dma(bo[:], bo_in[:])
                dma(onesb_t[:], onesb_in[:])
                for ch in range(NCH):
                    dma(wv[:, ch], r3(wv_in)[:, ch])
                dma_x(0)
                for ch in range(NCH):
                    dma(wq[:, ch], r3(wq_in)[:, ch])
                dma_x(1)
                for ch in range(NCH):
                    dma(wk[:, ch], r3(wk_in)[:, ch])
                dma_x(2)
                dma_x(3)
                for ch in range(NCH):
                    dma(wo[:, ch], r3(wo_in)[:, ch])
                # ones columns of V_aug: contiguous DMA to scratch, strided copy
                for vh in range(2):
                    nc.vector.tensor_copy(
                        (vv_a if vh == 0 else vv_b)[:]
                        .rearrange("p j (h e) -> p (j h) e", e=65)[:, :, 64:65],
                        onesb_t[:, vh * NJC * H // 2:(vh + 1) * NJC * H // 2]
                        .rearrange("p (n o) -> p n o", o=1))
                # zero qt up front (the dead half of the zero-half trick must
                # be zero to mask the other head's K rows in the shared kt;
                # live halves are overwritten by the Q projection drains)
                for h in range(H):
                    nc.gpsimd.memset(qt[:, h, :].bitcast(F32), 0.0)
                nc.gpsimd.partition_broadcast(bo_b[:], bo[:])

                def xtv(ch, start, size):
                    t = xts[start // 512]
                    off = start % 512
                    assert off + size <= 512
                    return t[:, ch, off:off + size]

                # ---- compute emission, ordered to match DMA arrival ----
                def emit_v(jcs):
                    for jc in jcs:
                        v_ps = ps.tile([128, NI], F32, tag="big")
                        for ch in range(NCH):
                            nc.tensor.matmul(v_ps[:, 0:MODEL],
                                             xtv(ch, jc * 128, 128),
                                             wv[:, ch],
                                             start=(ch == 0), stop=(ch == NCH - 1))
                        nc.vector.tensor_copy(
                            vvt(jc).rearrange("p (h e) -> p h e", e=65)[:, :, 0:64],
                            v_ps[:, 0:MODEL].rearrange("p (h e) -> p h e", e=64))

                def emit_q(hp, ib):
                    q_ps = ps.tile([128, NI], F32, tag="big")
                    for ch in range(NCH):
                        nc.tensor.matmul(
                            q_ps[:, 0:512],
                            wq[:, ch, hp * 128:(hp + 1) * 128],
                            xtv(ch, ib * 512, 512),
                            start=(ch == 0), stop=(ch == NCH - 1))
                    isl = slice(ib * 512, ib * 512 + 512)
                    nc.vector.tensor_scalar_add(
                        qt[0:64, 2 * hp, isl], q_ps[0:64, 0:512],
                        relb[0:64, hp:hp + 1])
                    nc.vector.tensor_scalar_add(
                        qt[64:128, 2 * hp + 1, isl], q_ps[64:128, 0:512],
                        relb[64:128, hp:hp + 1])

                def emit_k(hp, jb, sb):
                    k_ps = ps.tile([128, NI], F32, tag="big")
                    off = jb * NI + sb * 512
                    for ch in range(NCH):
                        nc.tensor.matmul(
                            k_ps[:, 0:512],
                            wk[:, ch, hp * 128:(hp + 1) * 128],
                            xtv(ch, off, 512),
                            start=(ch == 0), stop=(ch == NCH - 1))
                    jcs = slice(jb * 8 + sb * 4, jb * 8 + sb * 4 + 4)
                    src = k_ps[:, 0:512].rearrange("p (j m) -> p j m", m=128)
                    nc.vector.tensor_copy(kt[0:64, hp, jcs, :], src[0:64])
                    nc.vector.tensor_copy(kt[64:128, hp, jcs, :], src[64:128])

                emit_v(range(0, 1))
                emit_q(0, 0)
                emit_q(0, 1)
                for jb in range(2):
                    for sb in range(2):
                        emit_k(0, jb, sb)

                # ------ attention: flat pipeline over 128 S^T chunks (8 heads
                # x 16 j-chunks); PV lags LOOK chunks globally so it never
                # stalls on exp and head boundaries stay seamless; remaining
                # projections interleave at scheduled points to keep PE warm -
                ilv = {}
                for j in range(1, 16):      # V chunks 1..15 at g=1..15
                    ilv[j] = (lambda j=j: emit_v(range(j, j + 1)))
                gq = [17, 18, 44, 45, 66, 67]       # Q halves for hp 1..3
                gk = [20, 22, 24, 26,               # K quarters hp1
                      47, 49, 51, 53,               # hp2
                      69, 71, 73, 75]               # hp3
                for i, g in enumerate(gq):
                    hp, ib = 1 + i // 2, i % 2
                    ilv[g] = (lambda hp=hp, ib=ib: emit_q(hp, ib))
                for i, g in enumerate(gk):
                    hp, jb, sb = 1 + i // 4, (i // 2) % 2, i % 2
                    ilv[g] = (lambda hp=hp, jb=jb, sb=sb: emit_k(hp, jb, sb))
                with tc.tile_pool(name="pt", bufs=8) as ptp, \
                     tc.tile_pool(name="pv", bufs=1, space="PSUM") as pvp, \
                     tc.tile_pool(name="norm", bufs=2) as np_, \
                     tc.tile_pool(name="ysb", bufs=2) as yp_sb:
                    pts = {}
                    pvs = {}

                    def emit_st(g):
                        h, jc = g // NJC, g % NJC
                        st = ps.tile([128, NI], F32, tag="big")
                        for ih in range(2):
                            nc.tensor.matmul(
                                st[:, ih * 512:(ih + 1) * 512],
                                kt[:, h // 2, jc],
                                qt[:, h, ih * 512:(ih + 1) * 512],
                                start=True, stop=True)
                        pt = ptp.tile([128, NI], BF16, tag="pt")
                        pts[g] = pt
                        nc.scalar.activation(pt[:], st[:], EXP, scale=1.0)

                    def emit_pv(g):
                        h, jc = g // NJC, g % NJC
                        if jc == 0:
                            pv_t = pvp.tile([65, NI], F32, tag="pv", name="pv_t")
                            pvs[h] = pv_t
                        else:
                            pv_t = pvs[h]
                        pt = pts.pop(g)
                        for ih in range(2):
                            nc.tensor.matmul(
                                pv_t[:, ih * 512:(ih + 1) * 512],
                                vvt(jc)[:, h * 65:(h + 1) * 65],
                                pt[:, ih * 512:(ih + 1) * 512],
                                start=(jc == 0), stop=(jc == NJC - 1))
                        if jc == NJC - 1:
                            emit_norm(h)

                    def emit_norm(h):
                        hp, base = h // 2, (h % 2) * 64
                        pv_t = pvs.pop(h)
                        den = np_.tile([1, NI], F32, tag="den")
                        nc.vector.tensor_copy(den[:], pv_t[64:65, :])
                        rrow = np_.tile([1, NI], F32, tag="rrow")
                        nc.vector.reciprocal_approx_fast(rrow[:], den[:])
                        rb = np_.tile([64, NI], F32, tag="rb")
                        nc.gpsimd.partition_broadcast(rb[:], rrow[:])
                        nc.vector.tensor_tensor(
                            out=outt[base:base + 64, hp, :],
                            in0=pv_t[0:64, :], in1=rb[:],
                            op=mybir.AluOpType.mult)

                    for g in range(H * NJC + LOOK):
                        if g < H * NJC:
                            emit_st(g)
                        if g in ilv:
                            ilv[g]()
                        if g >= LOOK:
                            emit_pv(g - LOOK)

                    # ---------- output projection ----------
                    for ib in range(NI // 128):
                        y_ps = ps.tile([128, NI], F32, tag="big")
                        for ch in range(NCH):
                            nc.tensor.matmul(y_ps[:, 0:MODEL],
                                             outt[:, ch, ib * 128:(ib + 1) * 128],
                                             wo[:, ch],
                                             start=(ch == 0), stop=(ch == NCH - 1))
                        y_sb = yp_sb.tile([128, MODEL], F16, tag="ysb")
                        nc.vector.tensor_tensor(out=y_sb[:], in0=y_ps[:, 0:MODEL],
                                                in1=bo_b[:],
                                                op=mybir.AluOpType.add)
                        dma(y_out[ib * 128:(ib + 1) * 128, :], y_sb[:])

    nc.compile()
    return nc


def _get_compiled():
    global _COMPILED
    if _COMPILED is None:
        _COMPILED = _build()
    return _COMPILED


def kernel(x, Wq, Wk, Wv, Wo, bo, rel_content_bias, _trace=False):
    from concourse.bass_utils import run_bass_kernel_spmd
    import ml_dtypes

    nc = _get_compiled()
    BF = ml_dtypes.bfloat16

    x = np.asarray(x, dtype=np.float32)
    Wq = np.asarray(Wq, dtype=np.float32)
    Wk = np.asarray(Wk, dtype=np.float32)
    Wv = np.asarray(Wv, dtype=np.float32)
    Wo = np.asarray(Wo, dtype=np.float32)
    bo = np.asarray(bo, dtype=np.float32)
    bias = np.asarray(rel_content_bias, dtype=np.float32).reshape(H, DK)

    wq_b = (Wq * SCALE).astype(BF)
    wk_b = Wk.astype(BF)
    wv_b = Wv.astype(BF)
    # relb packed per head pair: rows 0:64 = even head bias, 64:128 = odd head
    relb = np.ascontiguousarray(
        bias.reshape(NHP, 2, DK).transpose(1, 2, 0).reshape(128, NHP))
    onesb = np.ones((128, NJC * H), BF)
    shared = {"wq": wq_b, "wk": wk_b, "wv": wv_b, "relb": relb, "wo": Wo,
              "bo": bo[None, :], "onesb": onesb}

    in_maps = []
    for c in range(8):
        b, half = c // 2, c % 2
        xt = np.ascontiguousarray(x[b].T)              # [512, 2048]
        if half:
            xt = np.ascontiguousarray(np.roll(xt, -NI, axis=1))
        in_maps.append({"xt": xt.astype(BF), **shared})

    res = run_bass_kernel_spmd(nc, in_maps, core_ids=list(range(8)),
                               trace=_trace)
    out = np.empty((B, N, MODEL), np.float32)
    for c in range(8):
        b, half = c // 2, c % 2
        out[b, half * NI:(half + 1) * NI, :] = res.results[c]["y"]
    if _trace:
        return out, res
    return out


# revision 23
# speedup vs baseline: 1.3726x; 1.0112x over previous
"""Trainium2 Bass kernel for multi-head attention (b=4, n=2048, d=512, h=8, dk=dv=64).

Sharding: 8 cores = 4 batches x 2 query-halves. Each core computes K/V for its
full batch sequence (2048) and attention outputs for its 1024 query rows.
No collectives needed; host stacks the per-core [1024, 512] outputs.

Per-core dataflow:
  x^T [512, 2048] staged in SBUF as bf16; projections (bf16 MMs, f32 PSUM)
  run mostly up front, emission-ordered against the HBM input stream.
  Q/K projections are head-PAIR packed: one [128 = h_even dims | h_odd dims]
  PSUM tile per pair covers two heads per moving stream (halved MM columns,
  unreplicated wq/wk).  Per-head S^T keeps full 128x128 stationaries via the
  zero-half trick: qt_h = [q+bias; 0] (or flipped), kt_h live half = K^T, dead
  half zeroed, so S^T = kt_h.T @ qt_h contracts the zero half away.  All
  PSUM->SBUF copies stay partition-aligned.  S^T/PV stay f32r/bf16 as in the
  baseline; exp on ScalarE from PSUM per [128,1024] chunk (the phase pacer),
  PV runs 3 chunks behind S^T so it never stalls on exp; K-projections for
  later pairs are interleaved mid-head to keep the PE HAM-warm.
"""
import numpy as np

B, N, MODEL = 4, 2048, 512
H, DK = 8, 64
SCALE = DK ** -0.5
NI = 1024           # query rows per core
NCH = MODEL // 128  # model-dim chunks
NJC = N // 128      # key/value chunks
NHP = H // 2        # head pairs
LOOK = 3            # PV chunk lookahead behind S^T

_COMPILED = None


def _build():
    import concourse.bass as bass
    from concourse import bacc
    import concourse.mybir as mybir
    import concourse.tile as tile

    F32 = mybir.dt.float32
    F32R = mybir.dt.float32r
    BF16 = mybir.dt.bfloat16
    F16 = mybir.dt.float16
    EXP = mybir.ActivationFunctionType.Exp

    nc = bacc.Bacc("TRN2", target_bir_lowering=False, debug=False, num_devices=8)
    xt_in = nc.dram_tensor("xt", [MODEL, N], BF16, kind="ExternalInput")
    wq_in = nc.dram_tensor("wq", [MODEL, MODEL], BF16, kind="ExternalInput")
    wk_in = nc.dram_tensor("wk", [MODEL, MODEL], BF16, kind="ExternalInput")
    wv_in = nc.dram_tensor("wv", [MODEL, MODEL], BF16, kind="ExternalInput")
    relb_in = nc.dram_tensor("relb", [128, NHP], F32, kind="ExternalInput")
    wo_in = nc.dram_tensor("wo", [MODEL, MODEL], F32R, kind="ExternalInput")
    bo_in = nc.dram_tensor("bo", [1, MODEL], F32, kind="ExternalInput")
    onesb_in = nc.dram_tensor("onesb", [128, NJC * H], BF16, kind="ExternalInput")
    y_out = nc.dram_tensor("y", [NI, MODEL], F16, kind="ExternalOutput")

    with tile.TileContext(nc) as tc:
        with (
            tc.tile_pool(name="w", bufs=1) as wp,
            tc.tile_pool(name="acts", bufs=1) as ap,
            tc.tile_pool(name="big", bufs=3, space="PSUM") as ps,
        ):
            # ---------- persistent tiles ----------
            wo = wp.tile([128, NCH, MODEL], F32R, tag="wo")
            bo = wp.tile([1, MODEL], F32, tag="bo")
            bo_b = wp.tile([128, MODEL], F32, tag="bo_b")
            vv_a = ap.tile([128, NJC // 2, H * 65], BF16, tag="vva")
            vv_b = ap.tile([128, NJC // 2, H * 65], BF16, tag="vvb")
            def vvt(jc):
                return (vv_a if jc < NJC // 2 else vv_b)[:, jc % (NJC // 2)]
            relb = ap.tile([128, NHP], F32, tag="relb")
            outt = ap.tile([128, NCH, NI], F32R, tag="outt")
            kt = ap.tile([128, NHP, NJC, 128], F32R, tag="kt")
            qt = ap.tile([128, H, NI], F32R, tag="qt")

            def r3(d):
                return d[:].rearrange("(c p) n -> p c n", p=128)

            dma_n = [0]
            def dma(out, in_):
                engs = (nc.sync, nc.gpsimd, nc.scalar)
                engs[dma_n[0] % 3].dma_start(out=out, in_=in_)
                dma_n[0] += 1

            with tc.tile_pool(name="proj", bufs=1) as pp:
                xt0 = pp.tile([128, NCH, 512], BF16, tag="xt0")
                xt1 = pp.tile([128, NCH, 512], BF16, tag="xt1")
                xt2 = pp.tile([128, NCH, 512], BF16, tag="xt2")
                xt3 = pp.tile([128, NCH, 512], BF16, tag="xt3")
                xts = [xt0, xt1, xt2, xt3]
                wq = pp.tile([128, NCH, MODEL], BF16, tag="wq")
                wk = pp.tile([128, NCH, MODEL], BF16, tag="wk")
                wv = pp.tile([128, NCH, MODEL], BF16, tag="wv")
                onesb_t = pp.tile([128, NJC * H], BF16, tag="onesb")

                # ---- DMA emission in global priority order (round-robin) ----
                xsrc = r3(xt_in)
                def dma_x(q):
                    for chh in range(2):
                        dma(xts[q][:, chh * 2:(chh + 1) * 2, :],
                            xsrc[:, chh * 2:(chh + 1) * 2, q * 512:(q + 1) * 512])
                dma(relb[:], relb_in[:])
                dma(bo[:], bo_in[:])
                dma(onesb_t[:], onesb_in[:])
                for ch in range(NCH):
                    dma(wv[:, ch], r3(wv_in)[:, ch])
                dma_x(0)
                for ch in range(NCH):
                    dma(wq[:, ch], r3(wq_in)[:, ch])
                dma_x(1)
                for ch in range(NCH):
                    dma(wk[:, ch], r3(wk_in)[:, ch])
                dma_x(2)
                dma_x(3)
                for ch in range(NCH):
                    dma(wo[:, ch], r3(wo_in)[:, ch])
                # ones columns of V_aug: contiguous DMA to scratch, strided copy
                for vh in range(2):
                    nc.vector.tensor_copy(
                        (vv_a if vh == 0 else vv_b)[:]
                        .rearrange("p j (h e) -> p (j h) e", e=65)[:, :, 64:65],
                        onesb_t[:, vh * NJC * H // 2:(vh + 1) * NJC * H // 2]
                        .rearrange("p (n o) -> p n o", o=1))
                # zero qt up front (the dead half of the zero-half trick must
                # be zero to mask the other head's K rows in the shared kt;
                # live halves are overwritten by the Q projection drains)
                for h in range(H):
                    nc.gpsimd.memset(qt[:, h, :].bitcast(F32), 0.0)
                nc.gpsimd.partition_broadcast(bo_b[:], bo[:])

                def xtv(ch, start, size):
                    t = xts[start // 512]
                    off = start % 512
                    assert off + size <= 512
                    return t[:, ch, off:off + size]

                # ---- compute emission, ordered to match DMA arrival ----
                def emit_v(jcs):
                    for jc in jcs:
                        v_ps = ps.tile([128, NI], F32, tag="big")
                        for ch in range(NCH):
                            nc.tensor.matmul(v_ps[:, 0:MODEL],
                                             xtv(ch, jc * 128, 128),
                                             wv[:, ch],
                                             start=(ch == 0), stop=(ch == NCH - 1))
                        nc.vector.tensor_copy(
                            vvt(jc).rearrange("p (h e) -> p h e", e=65)[:, :, 0:64],
                            v_ps[:, 0:MODEL].rearrange("p (h e) -> p h e", e=64))

                def emit_q(hp, ib):
                    q_ps = ps.tile([128, NI], F32, tag="big")
                    for ch in range(NCH):
                        nc.tensor.matmul(
                            q_ps[:, 0:512],
                            wq[:, ch, hp * 128:(hp + 1) * 128],
                            xtv(ch, ib * 512, 512),
                            start=(ch == 0), stop=(ch == NCH - 1))
                    isl = slice(ib * 512, ib * 512 + 512)
                    nc.vector.tensor_scalar_add(
                        qt[0:64, 2 * hp, isl], q_ps[0:64, 0:512],
                        relb[0:64, hp:hp + 1])
                    nc.vector.tensor_scalar_add(
                        qt[64:128, 2 * hp + 1, isl], q_ps[64:128, 0:512],
                        relb[64:128, hp:hp + 1])

                def emit_k(hp, jb, sb):
                    k_ps = ps.tile([128, NI], F32, tag="big")
                    off = jb * NI + sb * 512
                    for ch in range(NCH):
                        nc.tensor.matmul(
                            k_ps[:, 0:512],
                            wk[:, ch, hp * 128:(hp + 1) * 128],
                            xtv(ch, off, 512),
                            start=(ch == 0), stop=(ch == NCH - 1))
                    jcs = slice(jb * 8 + sb * 4, jb * 8 + sb * 4 + 4)
                    src = k_ps[:, 0:512].rearrange("p (j m) -> p j m", m=128)
                    nc.vector.tensor_copy(kt[0:64, hp, jcs, :], src[0:64])
                    nc.vector.tensor_copy(kt[64:128, hp, jcs, :], src[64:128])

                emit_v(range(0, 4))
                emit_q(0, 0)
                emit_q(0, 1)
                for jb in range(2):
                    for sb in range(2):
                        emit_k(0, jb, sb)

                # ------ attention: flat pipeline over 128 S^T chunks (8 heads
                # x 16 j-chunks); PV lags LOOK chunks globally so it never
                # stalls on exp and head boundaries stay seamless; remaining
                # projections interleave at scheduled points to keep PE warm -
                ilv = {}
                for j in range(4, 16):      # V chunks 4..15 at g=1..12
                    ilv[j - 3] = (lambda j=j: emit_v(range(j, j + 1)))
                gq = [14, 15, 40, 41, 64, 65]       # Q halves for hp 1..3
                gk = [17, 19, 21, 23,               # K quarters hp1
                      43, 45, 47, 49,               # hp2
                      67, 69, 71, 73]               # hp3
                for i, g in enumerate(gq):
                    hp, ib = 1 + i // 2, i % 2
                    ilv[g] = (lambda hp=hp, ib=ib: emit_q(hp, ib))
                for i, g in enumerate(gk):
                    hp, jb, sb = 1 + i // 4, (i // 2) % 2, i % 2
                    ilv[g] = (lambda hp=hp, jb=jb, sb=sb: emit_k(hp, jb, sb))
                gy = {0: [51, 53, 55, 57], 1: [75, 77, 79, 81],
                      2: [99, 101, 103, 105]}       # y-partials pairs 0..2
                for p, gs in gy.items():
                    for i, g in enumerate(gs):
                        assert g not in ilv
                        ilv[g] = (lambda p=p, i=i: emit_y(p, (2 * i, 2 * i + 1)))
                with tc.tile_pool(name="pt", bufs=8) as ptp, \
                     tc.tile_pool(name="pv", bufs=1, space="PSUM") as pvp, \
                     tc.tile_pool(name="norm", bufs=2) as np_, \
                     tc.tile_pool(name="yac", bufs=1) as yac, \
                     tc.tile_pool(name="ysb", bufs=2) as yp_sb:
                    pts = {}
                    pvs = {}
                    y_acc = yac.tile([128, NI // 128, MODEL], F32, tag="yacc")

                    def emit_y(p, ibs):
                        for ib in ibs:
                            y_ps = ps.tile([128, NI], F32, tag="big")
                            nc.tensor.matmul(
                                y_ps[:, 0:MODEL],
                                outt[:, p, ib * 128:(ib + 1) * 128],
                                wo[:, p], start=True, stop=True)
                            if p == 0:
                                nc.vector.tensor_tensor(
                                    out=y_acc[:, ib], in0=y_ps[:, 0:MODEL],
                                    in1=bo_b[:], op=mybir.AluOpType.add)
                            else:
                                nc.vector.tensor_tensor(
                                    out=y_acc[:, ib], in0=y_ps[:, 0:MODEL],
                                    in1=y_acc[:, ib], op=mybir.AluOpType.add)

                    def emit_st(g):
                        h, jc = g // NJC, g % NJC
                        st = ps.tile([128, NI], F32, tag="big")
                        for ih in range(2):
                            nc.tensor.matmul(
                                st[:, ih * 512:(ih + 1) * 512],
                                kt[:, h // 2, jc],
                                qt[:, h, ih * 512:(ih + 1) * 512],
                                start=True, stop=True)
                        pt = ptp.tile([128, NI], BF16, tag="pt")
                        pts[g] = pt
                        nc.scalar.activation(pt[:], st[:], EXP, scale=1.0)

                    def emit_pv(g):
                        h, jc = g // NJC, g % NJC
                        if jc == 0:
                            pv_t = pvp.tile([65, NI], F32, tag="pv", name="pv_t")
                            pvs[h] = pv_t
                        else:
                            pv_t = pvs[h]
                        pt = pts.pop(g)
                        for ih in range(2):
                            nc.tensor.matmul(
                                pv_t[:, ih * 512:(ih + 1) * 512],
                                vvt(jc)[:, h * 65:(h + 1) * 65],
                                pt[:, ih * 512:(ih + 1) * 512],
                                start=(jc == 0), stop=(jc == NJC - 1))
                        if jc == NJC - 1:
                            emit_norm(h)

                    def emit_norm(h):
                        hp, base = h // 2, (h % 2) * 64
                        pv_t = pvs.pop(h)
                        # drain PSUM fast so the next head's PV can start:
                        # copy numerator+denominator to SBUF, then normalize
                        # off-PSUM
                        den = np_.tile([1, NI], F32, tag="den")
                        nc.vector.tensor_copy(den[:], pv_t[64:65, :])
                        pvo = np_.tile([64, NI], F32, tag="pvo")
                        nc.vector.tensor_copy(pvo[:], pv_t[0:64, :])
                        rrow = np_.tile([1, NI], F32, tag="rrow")
                        nc.vector.reciprocal_approx_fast(rrow[:], den[:])
                        rb = np_.tile([64, NI], F32, tag="rb")
                        nc.gpsimd.partition_broadcast(rb[:], rrow[:])
                        nc.vector.tensor_tensor(
                            out=outt[base:base + 64, hp, :],
                            in0=pvo[:], in1=rb[:],
                            op=mybir.AluOpType.mult)

                    for g in range(H * NJC + LOOK):
                        if g < H * NJC:
                            emit_st(g)
                        if g in ilv:
                            ilv[g]()
                        if g >= LOOK:
                            emit_pv(g - LOOK)

                    # ------- output projection tail: pair 3 + writeback -----
                    for ib in range(NI // 128):
                        y_ps = ps.tile([128, NI], F32, tag="big")
                        nc.tensor.matmul(y_ps[:, 0:MODEL],
                                         outt[:, 3, ib * 128:(ib + 1) * 128],
                                         wo[:, 3], start=True, stop=True)
                        y_sb = yp_sb.tile([128, MODEL], F16, tag="ysb")
                        nc.vector.tensor_tensor(out=y_sb[:], in0=y_ps[:, 0:MODEL],
                                                in1=y_acc[:, ib],
                                                op=mybir.AluOpType.add)
                        dma(y_out[ib * 128:(ib + 1) * 128, :], y_sb[:])

    nc.compile()
    return nc


def _get_compiled():
    global _COMPILED
    if _COMPILED is None:
        _COMPILED = _build()
    return _COMPILED


def kernel(x, Wq, Wk, Wv, Wo, bo, rel_content_bias, _trace=False):
    from concourse.bass_utils import run_bass_kernel_spmd
    import ml_dtypes

    nc = _get_compiled()
    BF = ml_dtypes.bfloat16

    x = np.asarray(x, dtype=np.float32)
    Wq = np.asarray(Wq, dtype=np.float32)
    Wk = np.asarray(Wk, dtype=np.float32)
    Wv = np.asarray(Wv, dtype=np.float32)
    Wo = np.asarray(Wo, dtype=np.float32)
    bo = np.asarray(bo, dtype=np.float32)
    bias = np.asarray(rel_content_bias, dtype=np.float32).reshape(H, DK)

    wq_b = (Wq * SCALE).astype(BF)
    wk_b = Wk.astype(BF)
    wv_b = Wv.astype(BF)
    # relb packed per head pair: rows 0:64 = even head bias, 64:128 = odd head
    relb = np.ascontiguousarray(
        bias.reshape(NHP, 2, DK).transpose(1, 2, 0).reshape(128, NHP))
    onesb = np.ones((128, NJC * H), BF)
    shared = {"wq": wq_b, "wk": wk_b, "wv": wv_b, "relb": relb, "wo": Wo,
              "bo": bo[None, :], "onesb": onesb}

    in_maps = []
    for c in range(8):
        b, half = c // 2, c % 2
        xt = np.ascontiguousarray(x[b].T)              # [512, 2048]
        if half:
            xt = np.ascontiguousarray(np.roll(xt, -NI, axis=1))
        in_maps.append({"xt": xt.astype(BF), **shared})

    res = run_bass_kernel_spmd(nc, in_maps, core_ids=list(range(8)),
                               trace=_trace)
    out = np.empty((B, N, MODEL), np.float32)
    for c in range(8):
        b, half = c // 2, c % 2
        out[b, half * NI:(half + 1) * NI, :] = res.results[c]["y"]
    if _trace:
        return out, res
    return out


# revision 24
# speedup vs baseline: 1.4424x; 1.0509x over previous
"""Trainium2 Bass kernel for multi-head attention (b=4, n=2048, d=512, h=8, dk=dv=64).

Sharding: 8 cores = 4 batches x 2 query-halves. Each core computes K/V for its
full batch sequence (2048) and attention outputs for its 1024 query rows.
No collectives needed; host stacks the per-core [1024, 512] outputs.

Per-core dataflow:
  x^T [512, 2048] staged in SBUF as bf16; projections (bf16 MMs, f32 PSUM)
  run mostly up front, emission-ordered against the HBM input stream.
  Q/K projections are head-PAIR packed: one [128 = h_even dims | h_odd dims]
  PSUM tile per pair covers two heads per moving stream (halved MM columns,
  unreplicated wq/wk).  Per-head S^T keeps full 128x128 stationaries via the
  zero-half trick: qt_h = [q+bias; 0] (or flipped), kt_h live half = K^T, dead
  half zeroed, so S^T = kt_h.T @ qt_h contracts the zero half away.  All
  PSUM->SBUF copies stay partition-aligned.  S^T/PV stay f32r/bf16 as in the
  baseline; exp on ScalarE from PSUM per [128,1024] chunk (the phase pacer),
  PV runs 3 chunks behind S^T so it never stalls on exp; K-projections for
  later pairs are interleaved mid-head to keep the PE HAM-warm.
"""
import numpy as np

B, N, MODEL = 4, 2048, 512
H, DK = 8, 64
SCALE = DK ** -0.5
NI = 1024           # query rows per core
NCH = MODEL // 128  # model-dim chunks
NJC = N // 128      # key/value chunks
NHP = H // 2        # head pairs
LOOK = 3            # PV chunk lookahead behind S^T

_COMPILED = None


def _build():
    import concourse.bass as bass
    from concourse import bacc
    import concourse.mybir as mybir
    import concourse.tile as tile

    F32 = mybir.dt.float32
    F32R = mybir.dt.float32r
    BF16 = mybir.dt.bfloat16
    F16 = mybir.dt.float16
    EXP = mybir.ActivationFunctionType.Exp

    nc = bacc.Bacc("TRN2", target_bir_lowering=False, debug=False, num_devices=8)
    xt_in = nc.dram_tensor("xt", [MODEL, N], BF16, kind="ExternalInput")
    wq_in = nc.dram_tensor("wq", [MODEL, MODEL], BF16, kind="ExternalInput")
    wk_in = nc.dram_tensor("wk", [MODEL, MODEL], BF16, kind="ExternalInput")
    wv_in = nc.dram_tensor("wv", [MODEL, MODEL], BF16, kind="ExternalInput")
    relb_in = nc.dram_tensor("relb", [128, NHP], F32, kind="ExternalInput")
    wo_in = nc.dram_tensor("wo", [MODEL, MODEL], F32R, kind="ExternalInput")
    bo_in = nc.dram_tensor("bo", [1, MODEL], F32, kind="ExternalInput")
    onesb_in = nc.dram_tensor("onesb", [128, NJC * H], BF16, kind="ExternalInput")
    y_out = nc.dram_tensor("y", [NI, MODEL], F16, kind="ExternalOutput")

    with tile.TileContext(nc) as tc:
        with (
            tc.tile_pool(name="w", bufs=1) as wp,
            tc.tile_pool(name="acts", bufs=1) as ap,
            tc.tile_pool(name="big", bufs=2, space="PSUM") as ps,
            tc.tile_pool(name="qk", bufs=2, space="PSUM") as qkp,
        ):
            # ---------- persistent tiles ----------
            wo = wp.tile([128, NCH, MODEL], F32R, tag="wo")
            bo = wp.tile([1, MODEL], F32, tag="bo")
            bo_b = wp.tile([128, MODEL], F32, tag="bo_b")
            vv_a = ap.tile([128, NJC // 2, H * 65], BF16, tag="vva")
            vv_b = ap.tile([128, NJC // 2, H * 65], BF16, tag="vvb")
            def vvt(jc):
                return (vv_a if jc < NJC // 2 else vv_b)[:, jc % (NJC // 2)]
            relb = ap.tile([128, NHP], F32, tag="relb")
            outt = ap.tile([128, NCH, NI], F32R, tag="outt")
            kt = ap.tile([128, NHP, NJC, 128], F32R, tag="kt")
            qt = ap.tile([128, H, NI], F32R, tag="qt")

            def r3(d):
                return d[:].rearrange("(c p) n -> p c n", p=128)

            dma_n = [0]
            def dma(out, in_):
                engs = (nc.sync, nc.gpsimd, nc.scalar)
                engs[dma_n[0] % 3].dma_start(out=out, in_=in_)
                dma_n[0] += 1

            with tc.tile_pool(name="proj", bufs=1) as pp:
                xt0 = pp.tile([128, NCH, 512], BF16, tag="xt0")
                xt1 = pp.tile([128, NCH, 512], BF16, tag="xt1")
                xt2 = pp.tile([128, NCH, 512], BF16, tag="xt2")
                xt3 = pp.tile([128, NCH, 512], BF16, tag="xt3")
                xts = [xt0, xt1, xt2, xt3]
                wq = pp.tile([128, NCH, MODEL], BF16, tag="wq")
                wk = pp.tile([128, NCH, MODEL], BF16, tag="wk")
                wv = pp.tile([128, NCH, MODEL], BF16, tag="wv")
                onesb_t = pp.tile([128, NJC * H], BF16, tag="onesb")

                # ---- DMA emission in global priority order (round-robin) ----
                xsrc = r3(xt_in)
                def dma_x(q):
                    for chh in range(2):
                        dma(xts[q][:, chh * 2:(chh + 1) * 2, :],
                            xsrc[:, chh * 2:(chh + 1) * 2, q * 512:(q + 1) * 512])
                dma(relb[:], relb_in[:])
                dma(bo[:], bo_in[:])
                dma(onesb_t[:], onesb_in[:])
                for ch in range(NCH):
                    dma(wv[:, ch], r3(wv_in)[:, ch])
                dma_x(0)
                for ch in range(NCH):
                    dma(wq[:, ch], r3(wq_in)[:, ch])
                dma_x(1)
                for ch in range(NCH):
                    dma(wk[:, ch], r3(wk_in)[:, ch])
                dma_x(2)
                dma_x(3)
                for ch in range(NCH):
                    dma(wo[:, ch], r3(wo_in)[:, ch])
                # ones columns of V_aug: contiguous DMA to scratch, strided copy
                for vh in range(2):
                    nc.vector.tensor_copy(
                        (vv_a if vh == 0 else vv_b)[:]
                        .rearrange("p j (h e) -> p (j h) e", e=65)[:, :, 64:65],
                        onesb_t[:, vh * NJC * H // 2:(vh + 1) * NJC * H // 2]
                        .rearrange("p (n o) -> p n o", o=1))
                # zero qt up front (the dead half of the zero-half trick must
                # be zero to mask the other head's K rows in the shared kt;
                # live halves are overwritten by the Q projection drains)
                for h in range(H):
                    nc.gpsimd.memset(qt[:, h, :].bitcast(F32), 0.0)
                nc.gpsimd.partition_broadcast(bo_b[:], bo[:])

                def xtv(ch, start, size):
                    t = xts[start // 512]
                    off = start % 512
                    assert off + size <= 512
                    return t[:, ch, off:off + size]

                # ---- compute emission, ordered to match DMA arrival ----
                def emit_v(jcs):
                    for jc in jcs:
                        v_ps = qkp.tile([128, MODEL], F32, tag="qk")
                        for ch in range(NCH):
                            nc.tensor.matmul(v_ps[:],
                                             xtv(ch, jc * 128, 128),
                                             wv[:, ch],
                                             start=(ch == 0), stop=(ch == NCH - 1))
                        nc.vector.tensor_copy(
                            vvt(jc).rearrange("p (h e) -> p h e", e=65)[:, :, 0:64],
                            v_ps[:].rearrange("p (h e) -> p h e", e=64))

                def emit_q(hp, ib):
                    q_ps = qkp.tile([128, MODEL], F32, tag="qk")
                    for ch in range(NCH):
                        nc.tensor.matmul(
                            q_ps[:, 0:512],
                            wq[:, ch, hp * 128:(hp + 1) * 128],
                            xtv(ch, ib * 512, 512),
                            start=(ch == 0), stop=(ch == NCH - 1))
                    isl = slice(ib * 512, ib * 512 + 512)
                    nc.vector.tensor_scalar_add(
                        qt[0:64, 2 * hp, isl], q_ps[0:64, 0:512],
                        relb[0:64, hp:hp + 1])
                    nc.vector.tensor_scalar_add(
                        qt[64:128, 2 * hp + 1, isl], q_ps[64:128, 0:512],
                        relb[64:128, hp:hp + 1])

                def emit_k(hp, jb, sb):
                    k_ps = qkp.tile([128, MODEL], F32, tag="qk")
                    off = jb * NI + sb * 512
                    for ch in range(NCH):
                        nc.tensor.matmul(
                            k_ps[:, 0:512],
                            wk[:, ch, hp * 128:(hp + 1) * 128],
                            xtv(ch, off, 512),
                            start=(ch == 0), stop=(ch == NCH - 1))
                    jcs = slice(jb * 8 + sb * 4, jb * 8 + sb * 4 + 4)
                    src = k_ps[:, 0:512].rearrange("p (j m) -> p j m", m=128)
                    nc.vector.tensor_copy(kt[0:64, hp, jcs, :], src[0:64])
                    nc.vector.tensor_copy(kt[64:128, hp, jcs, :], src[64:128])

                emit_v(range(0, 4))
                emit_q(0, 0)
                emit_q(0, 1)
                for jb in range(2):
                    for sb in range(2):
                        emit_k(0, jb, sb)

                # ------ attention: flat pipeline over 128 S^T chunks (8 heads
                # x 16 j-chunks); PV lags LOOK chunks globally so it never
                # stalls on exp and head boundaries stay seamless; remaining
                # projections interleave at scheduled points to keep PE warm -
                ilv = {}
                for j in range(4, 16):      # V chunks 4..15 at g=1..12
                    ilv[j - 3] = (lambda j=j: emit_v(range(j, j + 1)))
                gq = [14, 15, 40, 41, 64, 65]       # Q halves for hp 1..3
                gk = [17, 19, 21, 23,               # K quarters hp1
                      43, 45, 47, 49,               # hp2
                      67, 69, 71, 73]               # hp3
                for i, g in enumerate(gq):
                    hp, ib = 1 + i // 2, i % 2
                    ilv[g] = (lambda hp=hp, ib=ib: emit_q(hp, ib))
                for i, g in enumerate(gk):
                    hp, jb, sb = 1 + i // 4, (i // 2) % 2, i % 2
                    ilv[g] = (lambda hp=hp, jb=jb, sb=sb: emit_k(hp, jb, sb))
                gy = {0: [51, 53, 55, 57], 1: [75, 77, 79, 81],
                      2: [99, 101, 103, 105]}       # y-partials pairs 0..2
                for p, gs in gy.items():
                    for i, g in enumerate(gs):
                        assert g not in ilv
                        ilv[g] = (lambda p=p, i=i: emit_y(p, (2 * i, 2 * i + 1)))
                with tc.tile_pool(name="pt", bufs=8) as ptp, \
                     tc.tile_pool(name="pv", bufs=1, space="PSUM") as pvp, \
                     tc.tile_pool(name="norm", bufs=2) as np_, \
                     tc.tile_pool(name="yac", bufs=1) as yac, \
                     tc.tile_pool(name="ysb", bufs=2) as yp_sb:
                    pts = {}
                    pvs = {}
                    y_acc = yac.tile([128, NI // 128, MODEL], F32, tag="yacc")

                    def emit_y(p, ibs):
                        for ib in ibs:
                            y_ps = qkp.tile([128, MODEL], F32, tag="qk")
                            nc.tensor.matmul(
                                y_ps[:],
                                outt[:, p, ib * 128:(ib + 1) * 128],
                                wo[:, p], start=True, stop=True)
                            if p == 0:
                                nc.vector.tensor_tensor(
                                    out=y_acc[:, ib], in0=y_ps[:],
                                    in1=bo_b[:], op=mybir.AluOpType.add)
                            else:
                                nc.vector.tensor_tensor(
                                    out=y_acc[:, ib], in0=y_ps[:],
                                    in1=y_acc[:, ib], op=mybir.AluOpType.add)

                    def emit_st(g):
                        h, jc = g // NJC, g % NJC
                        st = ps.tile([128, NI], F32, tag="big")
                        for ih in range(2):
                            nc.tensor.matmul(
                                st[:, ih * 512:(ih + 1) * 512],
                                kt[:, h // 2, jc],
                                qt[:, h, ih * 512:(ih + 1) * 512],
                                start=True, stop=True)
                        pt = ptp.tile([128, NI], BF16, tag="pt")
                        pts[g] = pt
                        nc.scalar.activation(pt[:], st[:], EXP, scale=1.0)

                    def emit_pv(g):
                        h, jc = g // NJC, g % NJC
                        if jc == 0:
                            pv_t = pvp.tile([65, NI], F32, tag="pv", name="pv_t")
                            pvs[h] = pv_t
                        else:
                            pv_t = pvs[h]
                        pt = pts.pop(g)
                        for ih in range(2):
                            nc.tensor.matmul(
                                pv_t[:, ih * 512:(ih + 1) * 512],
                                vvt(jc)[:, h * 65:(h + 1) * 65],
                                pt[:, ih * 512:(ih + 1) * 512],
                                start=(jc == 0), stop=(jc == NJC - 1))
                        if jc == NJC - 1:
                            emit_norm(h)

                    def emit_norm(h):
                        hp, base = h // 2, (h % 2) * 64
                        pv_t = pvs.pop(h)
                        # drain PSUM fast so the next head's PV can start:
                        # copy numerator+denominator to SBUF, then normalize
                        # off-PSUM
                        den = np_.tile([1, NI], F32, tag="den")
                        nc.vector.tensor_copy(den[:], pv_t[64:65, :])
                        pvo = np_.tile([64, NI], F32, tag="pvo")
                        nc.vector.tensor_copy(pvo[:], pv_t[0:64, :])
                        rrow = np_.tile([1, NI], F32, tag="rrow")
                        nc.vector.reciprocal_approx_fast(rrow[:], den[:])
                        rb = np_.tile([64, NI], F32, tag="rb")
                        nc.gpsimd.partition_broadcast(rb[:], rrow[:])
                        nc.vector.tensor_tensor(
                            out=outt[base:base + 64, hp, :],
                            in0=pvo[:], in1=rb[:],
                            op=mybir.AluOpType.mult)

                    for g in range(H * NJC + LOOK):
                        if g < H * NJC:
                            emit_st(g)
                        if g in ilv:
                            ilv[g]()
                        if g >= LOOK:
                            emit_pv(g - LOOK)

                    # ------- output projection tail: pair 3 + writeback -----
                    for ib in range(NI // 128):
                        y_ps = qkp.tile([128, MODEL], F32, tag="qk")
                        nc.tensor.matmul(y_ps[:],
                                         outt[:, 3, ib * 128:(ib + 1) * 128],
                                         wo[:, 3], start=True, stop=True)
                        y_sb = yp_sb.tile([128, MODEL], F16, tag="ysb")
                        nc.vector.tensor_tensor(out=y_sb[:], in0=y_ps[:],
                                                in1=y_acc[:, ib],
                                                op=mybir.AluOpType.add)
                        dma(y_out[ib * 128:(ib + 1) * 128, :], y_sb[:])

    nc.compile()
    return nc


def _get_compiled():
    global _COMPILED
    if _COMPILED is None:
        _COMPILED = _build()
    return _COMPILED


def kernel(x, Wq, Wk, Wv, Wo, bo, rel_content_bias, _trace=False):
    from concourse.bass_utils import run_bass_kernel_spmd
    import ml_dtypes

    nc = _get_compiled()
    BF = ml_dtypes.bfloat16

    x = np.asarray(x, dtype=np.float32)
    Wq = np.asarray(Wq, dtype=np.float32)
    Wk = np.asarray(Wk, dtype=np.float32)
    Wv = np.asarray(Wv, dtype=np.float32)
    Wo = np.asarray(Wo, dtype=np.float32)
    bo = np.asarray(bo, dtype=np.float32)
    bias = np.asarray(rel_content_bias, dtype=np.float32).reshape(H, DK)

    wq_b = (Wq * SCALE).astype(BF)
    wk_b = Wk.astype(BF)
    wv_b = Wv.astype(BF)
    # relb packed per head pair: rows 0:64 = even head bias, 64:128 = odd head
    relb = np.ascontiguousarray(
        bias.reshape(NHP, 2, DK).transpose(1, 2, 0).reshape(128, NHP))
    onesb = np.ones((128, NJC * H), BF)
    shared = {"wq": wq_b, "wk": wk_b, "wv": wv_b, "relb": relb, "wo": Wo,
              "bo": bo[None, :], "onesb": onesb}

    in_maps = []
    for c in range(8):
        b, half = c // 2, c % 2
        xt = np.ascontiguousarray(x[b].T)              # [512, 2048]
        if half:
            xt = np.ascontiguousarray(np.roll(xt, -NI, axis=1))
        in_maps.append({"xt": xt.astype(BF), **shared})

    res = run_bass_kernel_spmd(nc, in_maps, core_ids=list(range(8)),
                               trace=_trace)
    out = np.empty((B, N, MODEL), np.float32)
    for c in range(8):
        b, half = c // 2, c % 2
        out[b, half * NI:(half + 1) * NI, :] = res.results[c]["y"]
    if _trace:
        return out, res
    return out
